# revision 24
# baseline (speedup 1.0000x reference)
import sys, os
for _p in ("/opt/trn_rl_repo",):
    if _p not in sys.path:
        sys.path.append(_p)

import numpy as np
import ml_dtypes
from contextlib import ExitStack

import concourse.bass as bass
import concourse.bacc as bacc
import concourse.tile as tile
from concourse import mybir
from concourse.bass_utils import run_bass_kernel_spmd

F32 = mybir.dt.float32
BF16 = mybir.dt.bfloat16
BF_NP = ml_dtypes.bfloat16

DIM = 256
HEADS = 8
DIM_HEAD = 64
SLICE_NUM = 64
INNER = HEADS * DIM_HEAD  # 512
B, N = 4, 32768
NCORES = 8
NSHARD = N // 2  # 16384 tokens per core
P = 128
EXPF = mybir.ActivationFunctionType.Exp


def build_program(nshard, dbg=False):
    NT = nshard // P
    assert NT % 2 == 0
    nc = bacc.Bacc("TRN2", target_bir_lowering=False, debug=False,
                   num_devices=NCORES)
    if dbg:
        dbg_pooled = nc.dram_tensor("dbg_pooled", [P, 4, 130], F32,
                                    kind="ExternalOutput").ap()
        dbg_m2 = nc.dram_tensor("dbg_m2", [P, 4, DIM], BF16,
                                kind="ExternalOutput").ap()
        dbg_wT = nc.dram_tensor("dbg_wT", [P, 4, nshard], BF16,
                                kind="ExternalOutput").ap()
    xT_h = nc.dram_tensor("xT", [DIM, nshard], BF16, kind="ExternalInput")
    wfxT = nc.dram_tensor("wfxT", [DIM, INNER], BF16, kind="ExternalInput").ap()
    wlgT = nc.dram_tensor("wlgT", [DIM, INNER], BF16, kind="ExternalInput").ap()
    blg = nc.dram_tensor("blg", [1, INNER], BF16, kind="ExternalInput").ap()
    onesb = nc.dram_tensor("onesb", [1, P], BF16, kind="ExternalInput").ap()
    bfxb = nc.dram_tensor("bfxb", [P, 4, 64], F32, kind="ExternalInput").ap()
    wqT = nc.dram_tensor("wqT", [64, 64], F32, kind="ExternalInput").ap()
    wkT = nc.dram_tensor("wkT", [64, 64], F32, kind="ExternalInput").ap()
    wvT = nc.dram_tensor("wvT", [64, 64], F32, kind="ExternalInput").ap()
    woT = nc.dram_tensor("woT", [64, HEADS, DIM], F32, kind="ExternalInput").ap()
    bout8b = nc.dram_tensor("bout8b", [P, DIM], F32, kind="ExternalInput").ap()
    idf32 = nc.dram_tensor("idf32", [P, P], F32, kind="ExternalInput").ap()
    out_h = nc.dram_tensor("out", [nshard, DIM], F32, kind="ExternalOutput")
    out_ap = out_h.ap()

    with tile.TileContext(nc) as tc, ExitStack() as ctx:
        cpool = ctx.enter_context(tc.tile_pool(name="consts", bufs=1))
        big = ctx.enter_context(tc.tile_pool(name="big", bufs=1))

        # big weights on the scalar queue so x tiles start on sync at once
        wfx_sb = cpool.tile([P, 2, INNER], BF16)
        wlg_sb = cpool.tile([P, 2, INNER], BF16)
        for c in range(2):
            nc.scalar.dma_start(wfx_sb[:, c, :], wfxT[c * P:(c + 1) * P, :])
            nc.scalar.dma_start(wlg_sb[:, c, :], wlgT[c * P:(c + 1) * P, :])
        blg_sb = cpool.tile([1, INNER], BF16)
        nc.scalar.dma_start(blg_sb[:], blg[:])
        ones1 = cpool.tile([1, P], BF16)
        nc.scalar.dma_start(ones1[:], onesb[:])
        bfx_sb = cpool.tile([P, 4, 64], F32)
        nc.scalar.dma_start(bfx_sb[:], bfxb[:])
        wq_sb = cpool.tile([64, 64], F32)
        wk_sb = cpool.tile([64, 64], F32)
        wv_sb = cpool.tile([64, 64], F32)
        nc.scalar.dma_start(wq_sb[:], wqT[:])
        nc.scalar.dma_start(wk_sb[:], wkT[:])
        nc.scalar.dma_start(wv_sb[:], wvT[:])
        wo_sb = cpool.tile([64, HEADS, DIM], F32)
        nc.scalar.dma_start(wo_sb[:], woT[:])
        bout8_sb = cpool.tile([P, DIM], F32)
        nc.scalar.dma_start(bout8_sb[:], bout8b[:])
        idf_sb = cpool.tile([P, P], F32)
        nc.scalar.dma_start(idf_sb[:], idf32[:])

        # persistent across phases
        # transposed slice weights, blocked: [g, t4, (t%4)*4+c, tok]
        wT_sb = big.tile([P, nshard // (4 * P), 16, P], BF16)
        pooled_sb = big.tile([P, 4, 130], F32)   # after allreduce
        m2_sb = big.tile([P, 4, DIM], BF16)      # out_slice @ WoutT per hg
        # manual 4-slot fx staging; ones cols preset once (norm columns)
        FXS = 4
        fx2_sb = big.tile([P, FXS, 4, 130], BF16)
        nc.vector.memset(fx2_sb[:, :, :, 128:130], 1.0)
        # 8-slot w staging ring, consumed by pools + 4-wide batched transpose
        w8_sb = big.tile([P, 8, HEADS, SLICE_NUM], BF16)

        # ---------------- pass 1 ----------------
        # software-pipelined: pool matmuls + wT transpose for sub-tile t are
        # emitted DLY iterations late so the PE/sync queues never head-of-line
        # block on the exp->reduce->recip->mul chain.
        DLY = 3
        XB = 4  # sub-tiles per x DMA
        with tc.tile_pool(name="xp", bufs=2) as xpool, \
             tc.tile_pool(name="sp", bufs=8) as spool, \
             tc.tile_pool(name="fxps", bufs=2, space="PSUM") as fxps, \
             tc.tile_pool(name="lgps", bufs=2, space="PSUM") as lgps, \
             tc.tile_pool(name="poolps", bufs=1, space="PSUM") as poolps:
            # one PSUM bank per accumulator: accumulation groups must not
            # share a bank (start=True resets the bank's accum state)
            pool_ps = [poolps.tile([P, 130], F32, name=f"pool_ps{i}")
                       for i in range(4)]

            def emit_late(u):
                for q in range(4):
                    nc.tensor.matmul(pool_ps[q][:],
                                     w8_sb[:, u % 8, 2 * q:2 * q + 2, :],
                                     fx2_sb[:, u % FXS, q, :],
                                     start=(u == 0), stop=(u == NT - 1))
                if u % 4 == 3:
                    # one blocked DMA transpose for 4 sub-tiles:
                    # wT[g, (t',c), tok] = w[tok, (t',c)*128+g]
                    b = u // 4
                    nc.sync.dma_start_transpose(
                        wT_sb[:, b, :, :],
                        w8_sb[:, (b % 2) * 4:(b % 2) * 4 + 4, :, :])

            for t in range(NT):
                if t % XB == 0:
                    xt = xpool.tile([P, 2, XB * P], BF16)
                    src = bass.AP(xT_h, t * P,
                                  [[nshard, P], [P * nshard, 2], [1, XB * P]])
                    nc.sync.dma_start(xt[:], src)
                s = t % XB
                xa = xt[:, 0, s * P:(s + 1) * P]
                xb = xt[:, 1, s * P:(s + 1) * P]
                fxp = fxps.tile([P, 4, P], F32)
                nc.tensor.matmul(fxp[:], xa, wfx_sb[:, 0, :],
                                 start=True, stop=False)
                nc.tensor.matmul(fxp[:], xb, wfx_sb[:, 1, :],
                                 start=False, stop=True)
                lgp = lgps.tile([P, HEADS, SLICE_NUM], F32)
                nc.tensor.matmul(lgp[:], ones1[:], blg_sb[:],
                                 start=True, stop=False)
                nc.tensor.matmul(lgp[:], xa, wlg_sb[:, 0, :],
                                 start=False, stop=False)
                nc.tensor.matmul(lgp[:], xb, wlg_sb[:, 1, :],
                                 start=False, stop=True)
                # softmax over slices (bounded logits: skip max-sub)
                e_t = spool.tile([P, HEADS, SLICE_NUM], BF16)
                nc.scalar.activation(e_t[:], lgp[:], EXPF)
                nc.scalar.copy(fx2_sb[:, t % FXS, 0:2, 0:128], fxp[:, 0:2, :])
                s_t = spool.tile([P, HEADS], BF16)
                nc.vector.tensor_copy(fx2_sb[:, t % FXS, 2:4, 0:128],
                                      fxp[:, 2:4, :])
                with nc.allow_low_precision(
                        reason="softmax denom; bf16 noise ~0.4% ok here"):
                    nc.vector.tensor_reduce(s_t[:], e_t[:],
                                            axis=mybir.AxisListType.X,
                                            op=mybir.AluOpType.add)
                r_t = spool.tile([P, HEADS], F32)
                nc.vector.reciprocal(r_t[:], s_t[:])
                nc.gpsimd.tensor_mul(
                    w8_sb[:, t % 8, :, :], e_t[:],
                    r_t[:, :, None].to_broadcast([P, HEADS, SLICE_NUM]))
                if t >= DLY:
                    emit_late(t - DLY)
            for u in range(NT - DLY, NT):
                emit_late(u)

            # -------- allreduce pooled sums over the token-half pair --------
            with tc.tile_pool(name="ccdram", bufs=1, space="DRAM") as dpool:
                b_in = dpool.tile([P, 4, 130], F32)
                b_out = dpool.tile([P, 4, 130], F32)
                pre_sb = big.tile([P, 4, 130], F32)
                nc.scalar.copy(pre_sb[:, 0, :], pool_ps[0][:])
                nc.vector.tensor_copy(pre_sb[:, 1, :], pool_ps[1][:])
                nc.scalar.copy(pre_sb[:, 2, :], pool_ps[2][:])
                nc.vector.tensor_copy(pre_sb[:, 3, :], pool_ps[3][:])
                nc.sync.dma_start(b_in[:], pre_sb[:])
                nc.gpsimd.collective_compute(
                    "AllReduce", mybir.AluOpType.add,
                    replica_groups=[[0, 1], [2, 3], [4, 5], [6, 7]],
                    ins=[b_in.opt()], outs=[b_out.opt()])
                nc.sync.dma_start(pooled_sb[:], b_out[:])

        # ---------------- tiny slice attention (head pairs, 128-wide) ----
        with tc.tile_pool(name="mps", bufs=1, space="PSUM") as mps, \
             tc.tile_pool(name="msb", bufs=2) as msb:
            for q4 in range(4):
                # gather diagonal S blocks: row j*64+g <- own head's channels
                gath = msb.tile([P, 64], F32)
                for j in range(2):
                    nc.sync.dma_start(
                        gath[j * 64:(j + 1) * 64, :],
                        pooled_sb[j * 64:(j + 1) * 64, q4, j * 64:j * 64 + 64])
                norm = pooled_sb[:, q4, 128:129]
                nrm = msb.tile([P, 1], F32)
                nc.vector.tensor_scalar_add(nrm[:], norm, 1e-5)
                rho = msb.tile([P, 1], F32)
                nc.vector.reciprocal(rho[:], nrm[:])
                tmp = msb.tile([P, 64], F32)
                nc.vector.tensor_scalar_mul(tmp[:], bfx_sb[:, q4, :], norm)
                stp = msb.tile([P, 64], F32)
                nc.vector.tensor_add(stp[:], gath[:], tmp[:])
                st = msb.tile([P, 64], F32)
                nc.vector.tensor_scalar_mul(st[:], stp[:], rho[:])
                # stT [c, j*64+g] = slice_token[head j, g, c]
                stT_p = mps.tile([64, P], F32)
                nc.tensor.transpose(stT_p[:], st[:], idf_sb[:])
                stT = msb.tile([64, P], F32)
                nc.scalar.copy(stT[:], stT_p[:])
                # q^T, k^T for both heads at once: [o, j*64+g]
                qk_p = mps.tile([64, 2, P], F32)
                nc.tensor.matmul(qk_p[:, 0, :], wq_sb[:], stT[:],
                                 start=True, stop=True)
                nc.tensor.matmul(qk_p[:, 1, :], wk_sb[:], stT[:],
                                 start=True, stop=True)
                qk = msb.tile([64, 2, P], F32)
                nc.scalar.copy(qk[:], qk_p[:])
                # logits per head -> stacked [j*64+g, g']
                L_p = mps.tile([P, 64], F32)
                for j in range(2):
                    nc.tensor.matmul(L_p[j * 64:(j + 1) * 64, :],
                                     qk[:, 0, j * 64:(j + 1) * 64],
                                     qk[:, 1, j * 64:(j + 1) * 64],
                                     start=True, stop=True)
                ea = msb.tile([P, 64], F32)
                srow = msb.tile([P, 1], F32)
                nc.scalar.activation(ea[:], L_p[:], EXPF, accum_out=srow[:])
                rha = msb.tile([P, 1], F32)
                nc.vector.reciprocal(rha[:], srow[:])
                attn = msb.tile([P, 64], F32)
                nc.vector.tensor_scalar_mul(attn[:], ea[:], rha[:])
                # aT [g, j*64+g'] = attn[head j, g', g]
                aT_p = mps.tile([64, P], F32)
                nc.tensor.transpose(aT_p[:], attn[:], idf_sb[:])
                aT = msb.tile([64, P], F32)
                nc.scalar.copy(aT[:], aT_p[:])
                # v per head [g, o] (base partition 0), then os = attn @ v
                os_p = mps.tile([P, 64], F32)
                for j in range(2):
                    v_p = mps.tile([64, 64], F32)
                    nc.tensor.matmul(v_p[:], stT[:, j * 64:(j + 1) * 64],
                                     wv_sb[:], start=True, stop=True)
                    v_sb = msb.tile([64, 64], F32)
                    nc.scalar.copy(v_sb[:], v_p[:])
                    nc.tensor.matmul(os_p[j * 64:(j + 1) * 64, :],
                                     aT[:, j * 64:(j + 1) * 64], v_sb[:],
                                     start=True, stop=True)
                os_sb = msb.tile([P, 64], F32)
                nc.scalar.copy(os_sb[:], os_p[:])
                osT_p = mps.tile([64, P], F32)
                nc.tensor.transpose(osT_p[:], os_sb[:], idf_sb[:])
                osT = msb.tile([64, P], F32)
                nc.scalar.copy(osT[:], osT_p[:])
                m2_p = mps.tile([P, DIM], F32)
                for j in range(2):
                    nc.tensor.matmul(m2_p[j * 64:(j + 1) * 64, :],
                                     osT[:, j * 64:(j + 1) * 64],
                                     wo_sb[:, 2 * q4 + j, :],
                                     start=True, stop=True)
                # fold bout/8 into m2 (softmax weights sum to 8 over 512 g)
                nc.vector.tensor_add(m2_sb[:, q4, :], m2_p[:], bout8_sb[:])

        # ---------------- pass 2: unpool + output proj ----------------
        with tc.tile_pool(name="p2ps", bufs=8, space="PSUM") as p2ps, \
             tc.tile_pool(name="p2sb", bufs=4) as p2sb:
            for t4 in range(NT // 4):
                ob4 = p2sb.tile([P, 4, DIM], F32)
                for k in range(4):
                    t = 4 * t4 + k
                    op = p2ps.tile([P, DIM], F32)
                    for c in range(4):
                        nc.tensor.matmul(
                            op[:], wT_sb[:, t4, k * 4 + c, :],
                            m2_sb[:, c, :],
                            start=(c == 0), stop=(c == 3))
                    if k % 2 == 0:
                        nc.vector.tensor_copy(ob4[:, k, :], op[:])
                    else:
                        nc.scalar.copy(ob4[:, k, :], op[:])
                # one quad DMA: dst rows t4*512 + k*128 + p
                dst = bass.AP(out_h, t4 * 4 * P * DIM,
                              [[DIM, P], [P * DIM, 4], [1, DIM]])
                nc.sync.dma_start(dst, ob4[:])
        if dbg:
            nc.sync.dma_start(dbg_pooled[:], pooled_sb[:])
            nc.sync.dma_start(dbg_m2[:], m2_sb[:])
            nc.sync.dma_start(dbg_wT[:], wT_sb[:])
    nc.compile()
    return nc


def _bfx_pair(bfx):
    bfx2 = bfx.reshape(HEADS, DIM_HEAD)
    out = np.empty((P, 4, 64), np.float32)
    for q4 in range(4):
        for j in range(2):
            out[j * 64:(j + 1) * 64, q4, :] = bfx2[2 * q4 + j]
    return out


def prep_weights(inputs):
    f32 = np.float32
    Wfx = np.asarray(inputs["Wfx"], f32)
    bfx = np.asarray(inputs["bfx"], f32)
    Wx = np.asarray(inputs["Wx"], f32)
    bx = np.asarray(inputs["bx"], f32)
    Wslice = np.asarray(inputs["Wslice"], f32)
    bslice = np.asarray(inputs["bslice"], f32)
    tau = np.asarray(inputs["temperature"], f32).reshape(HEADS)
    Wq = np.asarray(inputs["Wq"], f32)
    Wk = np.asarray(inputs["Wk"], f32)
    Wv = np.asarray(inputs["Wv"], f32)
    Wout = np.asarray(inputs["Wout"], f32)
    bout = np.asarray(inputs["bout"], f32)

    wlg_blocks = []
    blg_blocks = []
    for h in range(HEADS):
        Wx_h = Wx[h * DIM_HEAD:(h + 1) * DIM_HEAD, :]
        bx_h = bx[h * DIM_HEAD:(h + 1) * DIM_HEAD]
        wlg_blocks.append((Wslice @ Wx_h) / tau[h])
        blg_blocks.append((Wslice @ bx_h + bslice) / tau[h])
    wlgT = np.ascontiguousarray(np.concatenate(wlg_blocks, 0).T, f32)
    blg = np.concatenate(blg_blocks, 0).reshape(1, INNER).astype(f32)
    scale = DIM_HEAD ** -0.5
    return {
        "wfxT": np.ascontiguousarray(Wfx.T).astype(BF_NP),
        "wlgT": wlgT.astype(BF_NP),
        "blg": blg.astype(BF_NP),
        "onesb": np.ones((1, P), BF_NP),
        "bfxb": _bfx_pair(bfx),
        "wqT": np.ascontiguousarray((Wq * scale).T, f32),
        "wkT": np.ascontiguousarray(Wk.T, f32),
        "wvT": np.ascontiguousarray(Wv.T, f32),
        "woT": np.ascontiguousarray(
            Wout.T.reshape(HEADS, DIM_HEAD, DIM).transpose(1, 0, 2), f32),
        "bout8b": np.ascontiguousarray(
            np.tile(bout[None, :] / 8.0, (P, 1)), f32),
        "idf32": np.eye(P, dtype=np.float32),
    }


_PROG = {}


def _get_prog(nshard, dbg=False):
    if (nshard, dbg) not in _PROG:
        _PROG[(nshard, dbg)] = build_program(nshard, dbg)
    return _PROG[(nshard, dbg)]


def run(inputs, nshard=NSHARD, trace=False, trace_cores=None, dbg=False):
    x = np.asarray(inputs["x"], np.float32)
    b_, n_, d_ = x.shape
    assert d_ == DIM and n_ == 2 * nshard and b_ == B
    nc = _get_prog(nshard, dbg)
    common = prep_weights(inputs)
    in_maps = []
    for core in range(NCORES):
        bb, half = core // 2, core % 2
        xs = x[bb, half * nshard:(half + 1) * nshard, :]
        m = dict(common)
        m["xT"] = np.ascontiguousarray(xs.T).astype(BF_NP)
        in_maps.append(m)
    res = run_bass_kernel_spmd(nc, in_maps, list(range(NCORES)),
                               trace=trace, trace_cores=trace_cores)
    full = np.empty((B, n_, DIM), np.float32)
    for core in range(NCORES):
        bb, half = core // 2, core % 2
        full[bb, half * nshard:(half + 1) * nshard, :] = \
            res.results[core]["out"]
    return full, res


def kernel(**inputs):
    out, _ = run(inputs)
    return out


# revision 26
# speedup vs baseline: 1.0481x; 1.0481x over previous
import sys, os
for _p in ("/opt/trn_rl_repo",):
    if _p not in sys.path:
        sys.path.append(_p)

import numpy as np
import ml_dtypes
from contextlib import ExitStack

import concourse.bass as bass
import concourse.bacc as bacc
import concourse.tile as tile
from concourse import mybir
from concourse.bass_utils import run_bass_kernel_spmd

F32 = mybir.dt.float32
BF16 = mybir.dt.bfloat16
BF_NP = ml_dtypes.bfloat16

DIM = 256
HEADS = 8
DIM_HEAD = 64
SLICE_NUM = 64
INNER = HEADS * DIM_HEAD  # 512
B, N = 4, 32768
NCORES = 8
NSHARD = N // 2  # 16384 tokens per core
P = 128
EXPF = mybir.ActivationFunctionType.Exp


def build_program(nshard, dbg=False):
    NT = nshard // P
    assert NT % 2 == 0
    nc = bacc.Bacc("TRN2", target_bir_lowering=False, debug=False,
                   num_devices=NCORES)
    if dbg:
        dbg_pooled = nc.dram_tensor("dbg_pooled", [P, 4, 130], F32,
                                    kind="ExternalOutput").ap()
        dbg_m2 = nc.dram_tensor("dbg_m2", [P, 4, DIM], BF16,
                                kind="ExternalOutput").ap()
        dbg_wT = nc.dram_tensor("dbg_wT", [P, 4, nshard], BF16,
                                kind="ExternalOutput").ap()
    xT_h = nc.dram_tensor("xT", [DIM, nshard], BF16, kind="ExternalInput")
    wfxT = nc.dram_tensor("wfxT", [DIM, INNER], BF16, kind="ExternalInput").ap()
    wlgT = nc.dram_tensor("wlgT", [DIM, INNER], BF16, kind="ExternalInput").ap()
    blg = nc.dram_tensor("blg", [1, INNER], BF16, kind="ExternalInput").ap()
    onesb = nc.dram_tensor("onesb", [1, P], BF16, kind="ExternalInput").ap()
    bfxb = nc.dram_tensor("bfxb", [P, 4, 64], F32, kind="ExternalInput").ap()
    wqT = nc.dram_tensor("wqT", [64, 64], F32, kind="ExternalInput").ap()
    wkT = nc.dram_tensor("wkT", [64, 64], F32, kind="ExternalInput").ap()
    wvT = nc.dram_tensor("wvT", [64, 64], F32, kind="ExternalInput").ap()
    woT = nc.dram_tensor("woT", [64, HEADS, DIM], F32, kind="ExternalInput").ap()
    bout8b = nc.dram_tensor("bout8b", [P, DIM], F32, kind="ExternalInput").ap()
    idf32 = nc.dram_tensor("idf32", [P, P], F32, kind="ExternalInput").ap()
    out_h = nc.dram_tensor("out", [nshard, DIM], F32, kind="ExternalOutput")
    out_ap = out_h.ap()

    with tile.TileContext(nc) as tc, ExitStack() as ctx:
        cpool = ctx.enter_context(tc.tile_pool(name="consts", bufs=1))
        big = ctx.enter_context(tc.tile_pool(name="big", bufs=1))

        # big weights on the scalar queue so x tiles start on sync at once
        wfx_sb = cpool.tile([P, 2, INNER], BF16)
        wlg_sb = cpool.tile([P, 2, INNER], BF16)
        for c in range(2):
            nc.scalar.dma_start(wfx_sb[:, c, :], wfxT[c * P:(c + 1) * P, :])
            nc.scalar.dma_start(wlg_sb[:, c, :], wlgT[c * P:(c + 1) * P, :])
        blg_sb = cpool.tile([1, INNER], BF16)
        nc.scalar.dma_start(blg_sb[:], blg[:])
        ones1 = cpool.tile([1, P], BF16)
        nc.scalar.dma_start(ones1[:], onesb[:])
        bfx_sb = cpool.tile([P, 4, 64], F32)
        nc.scalar.dma_start(bfx_sb[:], bfxb[:])
        wq_sb = cpool.tile([64, 64], F32)
        wk_sb = cpool.tile([64, 64], F32)
        wv_sb = cpool.tile([64, 64], F32)
        nc.scalar.dma_start(wq_sb[:], wqT[:])
        nc.scalar.dma_start(wk_sb[:], wkT[:])
        nc.scalar.dma_start(wv_sb[:], wvT[:])
        wo_sb = cpool.tile([64, HEADS, DIM], F32)
        nc.scalar.dma_start(wo_sb[:], woT[:])
        bout8_sb = cpool.tile([P, DIM], F32)
        nc.scalar.dma_start(bout8_sb[:], bout8b[:])
        idf_sb = cpool.tile([P, P], F32)
        nc.scalar.dma_start(idf_sb[:], idf32[:])

        # persistent across phases
        # transposed slice weights, blocked: [g, t4, (t%4)*4+c, tok]
        wT_sb = big.tile([P, nshard // (4 * P), 16, P], BF16)
        pooled_sb = big.tile([P, 4, 130], F32)   # after allreduce
        m2_sb = big.tile([P, 4, DIM], BF16)      # out_slice @ WoutT per hg
        # manual 4-slot fx staging; ones cols preset once (norm columns)
        FXS = 4
        fx2_sb = big.tile([P, FXS, 4, 130], BF16)
        nc.vector.memset(fx2_sb[:, :, :, 128:130], 1.0)
        # 8-slot w staging ring, consumed by pools + 4-wide batched transpose
        w8_sb = big.tile([P, 8, HEADS, SLICE_NUM], BF16)

        # ---------------- pass 1 ----------------
        # software-pipelined: pool matmuls + wT transpose for sub-tile t are
        # emitted DLY iterations late so the PE/sync queues never head-of-line
        # block on the exp->reduce->recip->mul chain.
        DLY = 3
        XB = 4  # sub-tiles per x DMA
        with tc.tile_pool(name="xp", bufs=2) as xpool, \
             tc.tile_pool(name="sp", bufs=8) as spool, \
             tc.tile_pool(name="fxps", bufs=2, space="PSUM") as fxps, \
             tc.tile_pool(name="lgps", bufs=2, space="PSUM") as lgps, \
             tc.tile_pool(name="poolps", bufs=1, space="PSUM") as poolps:
            # one PSUM bank per accumulator: accumulation groups must not
            # share a bank (start=True resets the bank's accum state)
            pool_ps = [poolps.tile([P, 130], F32, name=f"pool_ps{i}")
                       for i in range(4)]

            def emit_late(u):
                for q in range(4):
                    nc.tensor.matmul(pool_ps[q][:],
                                     w8_sb[:, u % 8, 2 * q:2 * q + 2, :],
                                     fx2_sb[:, u % FXS, q, :],
                                     start=(u == 0), stop=(u == NT - 1))
                if u % 4 == 3:
                    # one blocked DMA transpose for 4 sub-tiles:
                    # wT[g, (t',c), tok] = w[tok, (t',c)*128+g]
                    b = u // 4
                    nc.sync.dma_start_transpose(
                        wT_sb[:, b, :, :],
                        w8_sb[:, (b % 2) * 4:(b % 2) * 4 + 4, :, :])

            for t in range(NT):
                if t % XB == 0:
                    xt = xpool.tile([P, 2, XB * P], BF16)
                    src = bass.AP(xT_h, t * P,
                                  [[nshard, P], [P * nshard, 2], [1, XB * P]])
                    nc.sync.dma_start(xt[:], src)
                s = t % XB
                xa = xt[:, 0, s * P:(s + 1) * P]
                xb = xt[:, 1, s * P:(s + 1) * P]
                fxp = fxps.tile([P, 4, P], F32)
                nc.tensor.matmul(fxp[:], xa, wfx_sb[:, 0, :],
                                 start=True, stop=False)
                nc.tensor.matmul(fxp[:], xb, wfx_sb[:, 1, :],
                                 start=False, stop=True)
                lgp = lgps.tile([P, HEADS, SLICE_NUM], F32)
                nc.tensor.matmul(lgp[:], ones1[:], blg_sb[:],
                                 start=True, stop=False)
                nc.tensor.matmul(lgp[:], xa, wlg_sb[:, 0, :],
                                 start=False, stop=False)
                nc.tensor.matmul(lgp[:], xb, wlg_sb[:, 1, :],
                                 start=False, stop=True)
                # softmax over slices (bounded logits: skip max-sub)
                e_t = spool.tile([P, HEADS, SLICE_NUM], BF16)
                nc.scalar.activation(e_t[:], lgp[:], EXPF)
                nc.scalar.copy(fx2_sb[:, t % FXS, 0:2, 0:128], fxp[:, 0:2, :])
                s_t = spool.tile([P, HEADS], F32)
                nc.vector.tensor_copy(fx2_sb[:, t % FXS, 2:4, 0:128],
                                      fxp[:, 2:4, :])
                nc.vector.tensor_reduce(s_t[:], e_t[:],
                                        axis=mybir.AxisListType.X,
                                        op=mybir.AluOpType.add)
                r_t = spool.tile([P, HEADS], F32)
                nc.vector.reciprocal(r_t[:], s_t[:])
                nc.gpsimd.tensor_mul(
                    w8_sb[:, t % 8, :, :], e_t[:],
                    r_t[:, :, None].to_broadcast([P, HEADS, SLICE_NUM]))
                if t >= DLY:
                    emit_late(t - DLY)
            for u in range(NT - DLY, NT):
                emit_late(u)

            # -------- allreduce pooled sums over the token-half pair --------
            with tc.tile_pool(name="ccdram", bufs=1, space="DRAM") as dpool:
                b_in = dpool.tile([P, 4, 130], F32)
                b_out = dpool.tile([P, 4, 130], F32)
                pre_sb = big.tile([P, 4, 130], F32)
                nc.scalar.copy(pre_sb[:, 0, :], pool_ps[0][:])
                nc.vector.tensor_copy(pre_sb[:, 1, :], pool_ps[1][:])
                nc.scalar.copy(pre_sb[:, 2, :], pool_ps[2][:])
                nc.vector.tensor_copy(pre_sb[:, 3, :], pool_ps[3][:])
                nc.sync.dma_start(b_in[:], pre_sb[:])
                nc.gpsimd.collective_compute(
                    "AllReduce", mybir.AluOpType.add,
                    replica_groups=[[0, 1], [2, 3], [4, 5], [6, 7]],
                    ins=[b_in.opt()], outs=[b_out.opt()])
                nc.sync.dma_start(pooled_sb[:], b_out[:])

        # ---------------- tiny slice attention (head pairs, 128-wide) ----
        with tc.tile_pool(name="mps", bufs=1, space="PSUM") as mps, \
             tc.tile_pool(name="msb", bufs=2) as msb:
            for q4 in range(4):
                # gather diagonal S blocks: row j*64+g <- own head's channels
                gath = msb.tile([P, 64], F32)
                for j in range(2):
                    nc.sync.dma_start(
                        gath[j * 64:(j + 1) * 64, :],
                        pooled_sb[j * 64:(j + 1) * 64, q4, j * 64:j * 64 + 64])
                norm = pooled_sb[:, q4, 128:129]
                nrm = msb.tile([P, 1], F32)
                nc.vector.tensor_scalar_add(nrm[:], norm, 1e-5)
                rho = msb.tile([P, 1], F32)
                nc.vector.reciprocal(rho[:], nrm[:])
                tmp = msb.tile([P, 64], F32)
                nc.vector.tensor_scalar_mul(tmp[:], bfx_sb[:, q4, :], norm)
                stp = msb.tile([P, 64], F32)
                nc.vector.tensor_add(stp[:], gath[:], tmp[:])
                st = msb.tile([P, 64], F32)
                nc.vector.tensor_scalar_mul(st[:], stp[:], rho[:])
                # stT [c, j*64+g] = slice_token[head j, g, c]
                stT_p = mps.tile([64, P], F32)
                nc.tensor.transpose(stT_p[:], st[:], idf_sb[:])
                stT = msb.tile([64, P], F32)
                nc.scalar.copy(stT[:], stT_p[:])
                # q^T, k^T for both heads at once: [o, j*64+g]
                qk_p = mps.tile([64, 2, P], F32)
                nc.tensor.matmul(qk_p[:, 0, :], wq_sb[:], stT[:],
                                 start=True, stop=True)
                nc.tensor.matmul(qk_p[:, 1, :], wk_sb[:], stT[:],
                                 start=True, stop=True)
                qk = msb.tile([64, 2, P], F32)
                nc.scalar.copy(qk[:], qk_p[:])
                # logits per head -> stacked [j*64+g, g']
                L_p = mps.tile([P, 64], F32)
                for j in range(2):
                    nc.tensor.matmul(L_p[j * 64:(j + 1) * 64, :],
                                     qk[:, 0, j * 64:(j + 1) * 64],
                                     qk[:, 1, j * 64:(j + 1) * 64],
                                     start=True, stop=True)
                ea = msb.tile([P, 64], F32)
                srow = msb.tile([P, 1], F32)
                nc.scalar.activation(ea[:], L_p[:], EXPF, accum_out=srow[:])
                rha = msb.tile([P, 1], F32)
                nc.vector.reciprocal(rha[:], srow[:])
                attn = msb.tile([P, 64], F32)
                nc.vector.tensor_scalar_mul(attn[:], ea[:], rha[:])
                # aT [g, j*64+g'] = attn[head j, g', g]
                aT_p = mps.tile([64, P], F32)
                nc.tensor.transpose(aT_p[:], attn[:], idf_sb[:])
                aT = msb.tile([64, P], F32)
                nc.scalar.copy(aT[:], aT_p[:])
                # v per head [g, o] (base partition 0), then os = attn @ v
                os_p = mps.tile([P, 64], F32)
                for j in range(2):
                    v_p = mps.tile([64, 64], F32)
                    nc.tensor.matmul(v_p[:], stT[:, j * 64:(j + 1) * 64],
                                     wv_sb[:], start=True, stop=True)
                    v_sb = msb.tile([64, 64], F32)
                    nc.scalar.copy(v_sb[:], v_p[:])
                    nc.tensor.matmul(os_p[j * 64:(j + 1) * 64, :],
                                     aT[:, j * 64:(j + 1) * 64], v_sb[:],
                                     start=True, stop=True)
                os_sb = msb.tile([P, 64], F32)
                nc.scalar.copy(os_sb[:], os_p[:])
                osT_p = mps.tile([64, P], F32)
                nc.tensor.transpose(osT_p[:], os_sb[:], idf_sb[:])
                osT = msb.tile([64, P], F32)
                nc.scalar.copy(osT[:], osT_p[:])
                m2_p = mps.tile([P, DIM], F32)
                for j in range(2):
                    nc.tensor.matmul(m2_p[j * 64:(j + 1) * 64, :],
                                     osT[:, j * 64:(j + 1) * 64],
                                     wo_sb[:, 2 * q4 + j, :],
                                     start=True, stop=True)
                # fold bout/8 into m2 (softmax weights sum to 8 over 512 g)
                nc.vector.tensor_add(m2_sb[:, q4, :], m2_p[:], bout8_sb[:])

        # ---------------- pass 2: unpool + output proj ----------------
        with tc.tile_pool(name="p2ps", bufs=8, space="PSUM") as p2ps, \
             tc.tile_pool(name="p2sb", bufs=4) as p2sb:
            for t4 in range(NT // 4):
                ob4 = p2sb.tile([P, 4, DIM], F32)
                for k in range(4):
                    t = 4 * t4 + k
                    op = p2ps.tile([P, DIM], F32)
                    for c in range(4):
                        nc.tensor.matmul(
                            op[:], wT_sb[:, t4, k * 4 + c, :],
                            m2_sb[:, c, :],
                            start=(c == 0), stop=(c == 3))
                    if k % 2 == 0:
                        nc.vector.tensor_copy(ob4[:, k, :], op[:])
                    else:
                        nc.scalar.copy(ob4[:, k, :], op[:])
                # one quad DMA: dst rows t4*512 + k*128 + p
                dst = bass.AP(out_h, t4 * 4 * P * DIM,
                              [[DIM, P], [P * DIM, 4], [1, DIM]])
                nc.sync.dma_start(dst, ob4[:])
        if dbg:
            nc.sync.dma_start(dbg_pooled[:], pooled_sb[:])
            nc.sync.dma_start(dbg_m2[:], m2_sb[:])
            nc.sync.dma_start(dbg_wT[:], wT_sb[:])
    nc.compile()
    return nc


def _bfx_pair(bfx):
    bfx2 = bfx.reshape(HEADS, DIM_HEAD)
    out = np.empty((P, 4, 64), np.float32)
    for q4 in range(4):
        for j in range(2):
            out[j * 64:(j + 1) * 64, q4, :] = bfx2[2 * q4 + j]
    return out


def prep_weights(inputs):
    f32 = np.float32
    Wfx = np.asarray(inputs["Wfx"], f32)
    bfx = np.asarray(inputs["bfx"], f32)
    Wx = np.asarray(inputs["Wx"], f32)
    bx = np.asarray(inputs["bx"], f32)
    Wslice = np.asarray(inputs["Wslice"], f32)
    bslice = np.asarray(inputs["bslice"], f32)
    tau = np.asarray(inputs["temperature"], f32).reshape(HEADS)
    Wq = np.asarray(inputs["Wq"], f32)
    Wk = np.asarray(inputs["Wk"], f32)
    Wv = np.asarray(inputs["Wv"], f32)
    Wout = np.asarray(inputs["Wout"], f32)
    bout = np.asarray(inputs["bout"], f32)

    wlg_blocks = []
    blg_blocks = []
    for h in range(HEADS):
        Wx_h = Wx[h * DIM_HEAD:(h + 1) * DIM_HEAD, :]
        bx_h = bx[h * DIM_HEAD:(h + 1) * DIM_HEAD]
        wlg_blocks.append((Wslice @ Wx_h) / tau[h])
        blg_blocks.append((Wslice @ bx_h + bslice) / tau[h])
    wlgT = np.ascontiguousarray(np.concatenate(wlg_blocks, 0).T, f32)
    blg = np.concatenate(blg_blocks, 0).reshape(1, INNER).astype(f32)
    scale = DIM_HEAD ** -0.5
    return {
        "wfxT": np.ascontiguousarray(Wfx.T).astype(BF_NP),
        "wlgT": wlgT.astype(BF_NP),
        "blg": blg.astype(BF_NP),
        "onesb": np.ones((1, P), BF_NP),
        "bfxb": _bfx_pair(bfx),
        "wqT": np.ascontiguousarray((Wq * scale).T, f32),
        "wkT": np.ascontiguousarray(Wk.T, f32),
        "wvT": np.ascontiguousarray(Wv.T, f32),
        "woT": np.ascontiguousarray(
            Wout.T.reshape(HEADS, DIM_HEAD, DIM).transpose(1, 0, 2), f32),
        "bout8b": np.ascontiguousarray(
            np.tile(bout[None, :] / 8.0, (P, 1)), f32),
        "idf32": np.eye(P, dtype=np.float32),
    }


_PROG = {}


def _get_prog(nshard, dbg=False):
    if (nshard, dbg) not in _PROG:
        _PROG[(nshard, dbg)] = build_program(nshard, dbg)
    return _PROG[(nshard, dbg)]


def run(inputs, nshard=NSHARD, trace=False, trace_cores=None, dbg=False):
    x = np.asarray(inputs["x"], np.float32)
    b_, n_, d_ = x.shape
    assert d_ == DIM and n_ == 2 * nshard and b_ == B
    nc = _get_prog(nshard, dbg)
    common = prep_weights(inputs)
    in_maps = []
    for core in range(NCORES):
        bb, half = core // 2, core % 2
        xs = x[bb, half * nshard:(half + 1) * nshard, :]
        m = dict(common)
        m["xT"] = np.ascontiguousarray(xs.T).astype(BF_NP)
        in_maps.append(m)
    res = run_bass_kernel_spmd(nc, in_maps, list(range(NCORES)),
                               trace=trace, trace_cores=trace_cores)
    full = np.empty((B, n_, DIM), np.float32)
    for core in range(NCORES):
        bb, half = core // 2, core % 2
        full[bb, half * nshard:(half + 1) * nshard, :] = \
            res.results[core]["out"]
    return full, res


def kernel(**inputs):
    out, _ = run(inputs)
    return out


# revision 30
# speedup vs baseline: 1.0759x; 1.0266x over previous
import sys, os
for _p in ("/opt/trn_rl_repo",):
    if _p not in sys.path:
        sys.path.append(_p)

import numpy as np
import ml_dtypes
from contextlib import ExitStack

import concourse.bass as bass
import concourse.bacc as bacc
import concourse.tile as tile
from concourse import mybir
from concourse.bass_utils import run_bass_kernel_spmd

F32 = mybir.dt.float32
BF16 = mybir.dt.bfloat16
BF_NP = ml_dtypes.bfloat16

DIM = 256
HEADS = 8
DIM_HEAD = 64
SLICE_NUM = 64
INNER = HEADS * DIM_HEAD  # 512
B, N = 4, 32768
NCORES = 8
NSHARD = N // 2  # 16384 tokens per core
P = 128
EXPF = mybir.ActivationFunctionType.Exp


def build_program(nshard, dbg=False):
    NT = nshard // P
    assert NT % 2 == 0
    nc = bacc.Bacc("TRN2", target_bir_lowering=False, debug=False,
                   num_devices=NCORES)
    if dbg:
        dbg_pooled = nc.dram_tensor("dbg_pooled", [P, 4, 130], F32,
                                    kind="ExternalOutput").ap()
        dbg_m2 = nc.dram_tensor("dbg_m2", [P, 4, DIM], BF16,
                                kind="ExternalOutput").ap()
        dbg_wT = nc.dram_tensor("dbg_wT", [P, 4, nshard], BF16,
                                kind="ExternalOutput").ap()
    xT_h = nc.dram_tensor("xT", [DIM, nshard], BF16, kind="ExternalInput")
    wfxT = nc.dram_tensor("wfxT", [DIM, INNER], BF16, kind="ExternalInput").ap()
    wlgT = nc.dram_tensor("wlgT", [DIM, INNER], BF16, kind="ExternalInput").ap()
    blg = nc.dram_tensor("blg", [1, INNER], BF16, kind="ExternalInput").ap()
    onesb = nc.dram_tensor("onesb", [1, P], BF16, kind="ExternalInput").ap()
    bfxb = nc.dram_tensor("bfxb", [P, 4, 64], F32, kind="ExternalInput").ap()
    wqT = nc.dram_tensor("wqT", [64, 64], F32, kind="ExternalInput").ap()
    wkT = nc.dram_tensor("wkT", [64, 64], F32, kind="ExternalInput").ap()
    wvT = nc.dram_tensor("wvT", [64, 64], F32, kind="ExternalInput").ap()
    woT = nc.dram_tensor("woT", [64, HEADS, DIM], F32, kind="ExternalInput").ap()
    bout8b = nc.dram_tensor("bout8b", [P, DIM], F32, kind="ExternalInput").ap()
    idf32 = nc.dram_tensor("idf32", [P, P], F32, kind="ExternalInput").ap()
    out_h = nc.dram_tensor("out", [nshard, DIM], F32, kind="ExternalOutput")
    out_ap = out_h.ap()

    with tile.TileContext(nc) as tc, ExitStack() as ctx:
        cpool = ctx.enter_context(tc.tile_pool(name="consts", bufs=1))
        big = ctx.enter_context(tc.tile_pool(name="big", bufs=1))

        # big weights on the scalar queue so x tiles start on sync at once
        wfx_sb = cpool.tile([P, 2, INNER], BF16)
        wlg_sb = cpool.tile([P, 2, INNER], BF16)
        for c in range(2):
            nc.scalar.dma_start(wfx_sb[:, c, :], wfxT[c * P:(c + 1) * P, :])
            nc.scalar.dma_start(wlg_sb[:, c, :], wlgT[c * P:(c + 1) * P, :])
        blg_sb = cpool.tile([1, INNER], BF16)
        nc.scalar.dma_start(blg_sb[:], blg[:])
        ones1 = cpool.tile([1, P], BF16)
        nc.scalar.dma_start(ones1[:], onesb[:])
        bfx_sb = cpool.tile([P, 4, 64], F32)
        nc.scalar.dma_start(bfx_sb[:], bfxb[:])
        wq_sb = cpool.tile([64, 64], F32)
        wk_sb = cpool.tile([64, 64], F32)
        wv_sb = cpool.tile([64, 64], F32)
        nc.scalar.dma_start(wq_sb[:], wqT[:])
        nc.scalar.dma_start(wk_sb[:], wkT[:])
        nc.scalar.dma_start(wv_sb[:], wvT[:])
        wo_sb = cpool.tile([64, HEADS, DIM], F32)
        nc.scalar.dma_start(wo_sb[:], woT[:])
        bout8_sb = cpool.tile([P, DIM], F32)
        nc.scalar.dma_start(bout8_sb[:], bout8b[:])
        idf_sb = cpool.tile([P, P], F32)
        nc.scalar.dma_start(idf_sb[:], idf32[:])

        # persistent across phases
        # transposed slice weights, blocked: [g, t4, (t%4)*4+c, tok]
        wT_sb = big.tile([P, nshard // (4 * P), 16, P], BF16)
        pooled_sb = big.tile([P, 4, 130], F32)   # after allreduce
        m2_sb = big.tile([P, 4, DIM], BF16)      # out_slice @ WoutT per hg
        # manual 4-slot fx staging; ones cols preset once (norm columns)
        FXS = 4
        fx2_sb = big.tile([P, FXS, 4, 130], BF16)
        nc.vector.memset(fx2_sb[:, :, :, 128:130], 1.0)
        # 8-slot w staging ring, consumed by pools + 4-wide batched transpose
        w8_sb = big.tile([P, 8, HEADS, SLICE_NUM], BF16)

        # ---------------- pass 1 ----------------
        # software-pipelined: pool matmuls + wT transpose for sub-tile t are
        # emitted DLY iterations late so the PE/sync queues never head-of-line
        # block on the exp->reduce->recip->mul chain.
        DLY = 3
        XB = 4  # sub-tiles per x DMA
        with tc.tile_pool(name="xp", bufs=2) as xpool, \
             tc.tile_pool(name="sp", bufs=8) as spool, \
             tc.tile_pool(name="fxps", bufs=3, space="PSUM") as fxps, \
             tc.tile_pool(name="lgps", bufs=3, space="PSUM") as lgps, \
             tc.tile_pool(name="poolps", bufs=1, space="PSUM") as poolps:
            # two accumulators per bank; start=True resets per-address, so
            # disjoint column ranges in one bank are safe
            pool_ps = [poolps.tile([P, 2, 130], F32, name=f"pool_ps{i}")
                       for i in range(2)]

            def emit_late(u):
                for q in range(4):
                    nc.tensor.matmul(pool_ps[q // 2][:, q % 2, :],
                                     w8_sb[:, u % 8, 2 * q:2 * q + 2, :],
                                     fx2_sb[:, u % FXS, q, :],
                                     start=(u == 0), stop=(u == NT - 1))
                if u % 4 == 3:
                    # one blocked DMA transpose for 4 sub-tiles:
                    # wT[g, (t',c), tok] = w[tok, (t',c)*128+g]
                    b = u // 4
                    nc.sync.dma_start_transpose(
                        wT_sb[:, b, :, :],
                        w8_sb[:, (b % 2) * 4:(b % 2) * 4 + 4, :, :])

            for t in range(NT):
                if t % XB == 0:
                    xt = xpool.tile([P, 2, XB * P], BF16)
                    src = bass.AP(xT_h, t * P,
                                  [[nshard, P], [P * nshard, 2], [1, XB * P]])
                    nc.sync.dma_start(xt[:], src)
                s = t % XB
                xa = xt[:, 0, s * P:(s + 1) * P]
                xb = xt[:, 1, s * P:(s + 1) * P]
                fxp = fxps.tile([P, 4, P], F32)
                nc.tensor.matmul(fxp[:], xa, wfx_sb[:, 0, :],
                                 start=True, stop=False)
                nc.tensor.matmul(fxp[:], xb, wfx_sb[:, 1, :],
                                 start=False, stop=True)
                lgp = lgps.tile([P, HEADS, SLICE_NUM], F32)
                nc.tensor.matmul(lgp[:], ones1[:], blg_sb[:],
                                 start=True, stop=False)
                nc.tensor.matmul(lgp[:], xa, wlg_sb[:, 0, :],
                                 start=False, stop=False)
                nc.tensor.matmul(lgp[:], xb, wlg_sb[:, 1, :],
                                 start=False, stop=True)
                # softmax over slices (bounded logits: skip max-sub)
                nc.scalar.copy(fx2_sb[:, t % FXS, 0:2, 0:128], fxp[:, 0:2, :])
                e_t = spool.tile([P, HEADS, SLICE_NUM], BF16)
                nc.scalar.activation(e_t[:], lgp[:], EXPF)
                s_t = spool.tile([P, HEADS], F32)
                nc.vector.tensor_copy(fx2_sb[:, t % FXS, 2:4, 0:128],
                                      fxp[:, 2:4, :])
                nc.vector.tensor_reduce(s_t[:], e_t[:],
                                        axis=mybir.AxisListType.X,
                                        op=mybir.AluOpType.add)
                r_t = spool.tile([P, HEADS], F32)
                nc.vector.reciprocal(r_t[:], s_t[:])
                nc.gpsimd.tensor_mul(
                    w8_sb[:, t % 8, :, :], e_t[:],
                    r_t[:, :, None].to_broadcast([P, HEADS, SLICE_NUM]))
                if t >= DLY:
                    emit_late(t - DLY)
            for u in range(NT - DLY, NT):
                emit_late(u)

            # -------- allreduce pooled sums over the token-half pair --------
            with tc.tile_pool(name="ccdram", bufs=1, space="DRAM") as dpool:
                b_in = dpool.tile([P, 4, 130], BF16)
                b_out = dpool.tile([P, 4, 130], BF16)
                pre_sb = big.tile([P, 4, 130], BF16)
                nc.scalar.copy(pre_sb[:, 0:2, :], pool_ps[0][:])
                nc.vector.tensor_copy(pre_sb[:, 2:4, :], pool_ps[1][:])
                nc.sync.dma_start(b_in[:], pre_sb[:])
                nc.gpsimd.collective_compute(
                    "AllReduce", mybir.AluOpType.add,
                    replica_groups=[[0, 1], [2, 3], [4, 5], [6, 7]],
                    ins=[b_in.opt()], outs=[b_out.opt()])
                pooled_bf = big.tile([P, 4, 130], BF16)
                nc.sync.dma_start(pooled_bf[:], b_out[:])
                nc.vector.tensor_copy(pooled_sb[:], pooled_bf[:])

        # ---------------- tiny slice attention (head pairs, 128-wide) ----
        with tc.tile_pool(name="mps", bufs=1, space="PSUM") as mps, \
             tc.tile_pool(name="msb", bufs=2) as msb:
            for q4 in range(4):
                # gather diagonal S blocks: row j*64+g <- own head's channels
                gath = msb.tile([P, 64], F32)
                for j in range(2):
                    nc.sync.dma_start(
                        gath[j * 64:(j + 1) * 64, :],
                        pooled_sb[j * 64:(j + 1) * 64, q4, j * 64:j * 64 + 64])
                norm = pooled_sb[:, q4, 128:129]
                nrm = msb.tile([P, 1], F32)
                nc.vector.tensor_scalar_add(nrm[:], norm, 1e-5)
                rho = msb.tile([P, 1], F32)
                nc.vector.reciprocal(rho[:], nrm[:])
                tmp = msb.tile([P, 64], F32)
                nc.vector.tensor_scalar_mul(tmp[:], bfx_sb[:, q4, :], norm)
                stp = msb.tile([P, 64], F32)
                nc.vector.tensor_add(stp[:], gath[:], tmp[:])
                st = msb.tile([P, 64], F32)
                nc.vector.tensor_scalar_mul(st[:], stp[:], rho[:])
                # stT [c, j*64+g] = slice_token[head j, g, c]
                stT_p = mps.tile([64, P], F32)
                nc.tensor.transpose(stT_p[:], st[:], idf_sb[:])
                stT = msb.tile([64, P], F32)
                nc.scalar.copy(stT[:], stT_p[:])
                # q^T, k^T for both heads at once: [o, j*64+g]
                qk_p = mps.tile([64, 2, P], F32)
                nc.tensor.matmul(qk_p[:, 0, :], wq_sb[:], stT[:],
                                 start=True, stop=True)
                nc.tensor.matmul(qk_p[:, 1, :], wk_sb[:], stT[:],
                                 start=True, stop=True)
                qk = msb.tile([64, 2, P], F32)
                nc.scalar.copy(qk[:], qk_p[:])
                # logits per head -> stacked [j*64+g, g']
                L_p = mps.tile([P, 64], F32)
                for j in range(2):
                    nc.tensor.matmul(L_p[j * 64:(j + 1) * 64, :],
                                     qk[:, 0, j * 64:(j + 1) * 64],
                                     qk[:, 1, j * 64:(j + 1) * 64],
                                     start=True, stop=True)
                ea = msb.tile([P, 64], F32)
                srow = msb.tile([P, 1], F32)
                nc.scalar.activation(ea[:], L_p[:], EXPF, accum_out=srow[:])
                rha = msb.tile([P, 1], F32)
                nc.vector.reciprocal(rha[:], srow[:])
                attn = msb.tile([P, 64], F32)
                nc.vector.tensor_scalar_mul(attn[:], ea[:], rha[:])
                # aT [g, j*64+g'] = attn[head j, g', g]
                aT_p = mps.tile([64, P], F32)
                nc.tensor.transpose(aT_p[:], attn[:], idf_sb[:])
                aT = msb.tile([64, P], F32)
                nc.scalar.copy(aT[:], aT_p[:])
                # v per head [g, o] (base partition 0), then os = attn @ v
                os_p = mps.tile([P, 64], F32)
                for j in range(2):
                    v_p = mps.tile([64, 64], F32)
                    nc.tensor.matmul(v_p[:], stT[:, j * 64:(j + 1) * 64],
                                     wv_sb[:], start=True, stop=True)
                    v_sb = msb.tile([64, 64], F32)
                    nc.scalar.copy(v_sb[:], v_p[:])
                    nc.tensor.matmul(os_p[j * 64:(j + 1) * 64, :],
                                     aT[:, j * 64:(j + 1) * 64], v_sb[:],
                                     start=True, stop=True)
                os_sb = msb.tile([P, 64], F32)
                nc.scalar.copy(os_sb[:], os_p[:])
                osT_p = mps.tile([64, P], F32)
                nc.tensor.transpose(osT_p[:], os_sb[:], idf_sb[:])
                osT = msb.tile([64, P], F32)
                nc.scalar.copy(osT[:], osT_p[:])
                m2_p = mps.tile([P, DIM], F32)
                for j in range(2):
                    nc.tensor.matmul(m2_p[j * 64:(j + 1) * 64, :],
                                     osT[:, j * 64:(j + 1) * 64],
                                     wo_sb[:, 2 * q4 + j, :],
                                     start=True, stop=True)
                # fold bout/8 into m2 (softmax weights sum to 8 over 512 g)
                nc.vector.tensor_add(m2_sb[:, q4, :], m2_p[:], bout8_sb[:])

        # ---------------- pass 2: unpool + output proj ----------------
        with tc.tile_pool(name="p2ps", bufs=8, space="PSUM") as p2ps, \
             tc.tile_pool(name="p2sb", bufs=5) as p2sb:
            for t4 in range(NT // 4):
                ob4 = p2sb.tile([P, 4, DIM], F32)
                for k in range(4):
                    t = 4 * t4 + k
                    op = p2ps.tile([P, DIM], F32)
                    for c in range(4):
                        nc.tensor.matmul(
                            op[:], wT_sb[:, t4, k * 4 + c, :],
                            m2_sb[:, c, :],
                            start=(c == 0), stop=(c == 3))
                    if k % 2 == 0:
                        nc.vector.tensor_copy(ob4[:, k, :], op[:])
                    else:
                        nc.scalar.copy(ob4[:, k, :], op[:])
                # one quad DMA: dst rows t4*512 + k*128 + p
                dst = bass.AP(out_h, t4 * 4 * P * DIM,
                              [[DIM, P], [P * DIM, 4], [1, DIM]])
                nc.sync.dma_start(dst, ob4[:])
        if dbg:
            nc.sync.dma_start(dbg_pooled[:], pooled_sb[:])
            nc.sync.dma_start(dbg_m2[:], m2_sb[:])
            nc.sync.dma_start(dbg_wT[:], wT_sb[:])
    nc.compile()
    return nc


def _bfx_pair(bfx):
    bfx2 = bfx.reshape(HEADS, DIM_HEAD)
    out = np.empty((P, 4, 64), np.float32)
    for q4 in range(4):
        for j in range(2):
            out[j * 64:(j + 1) * 64, q4, :] = bfx2[2 * q4 + j]
    return out


def prep_weights(inputs):
    f32 = np.float32
    Wfx = np.asarray(inputs["Wfx"], f32)
    bfx = np.asarray(inputs["bfx"], f32)
    Wx = np.asarray(inputs["Wx"], f32)
    bx = np.asarray(inputs["bx"], f32)
    Wslice = np.asarray(inputs["Wslice"], f32)
    bslice = np.asarray(inputs["bslice"], f32)
    tau = np.asarray(inputs["temperature"], f32).reshape(HEADS)
    Wq = np.asarray(inputs["Wq"], f32)
    Wk = np.asarray(inputs["Wk"], f32)
    Wv = np.asarray(inputs["Wv"], f32)
    Wout = np.asarray(inputs["Wout"], f32)
    bout = np.asarray(inputs["bout"], f32)

    wlg_blocks = []
    blg_blocks = []
    for h in range(HEADS):
        Wx_h = Wx[h * DIM_HEAD:(h + 1) * DIM_HEAD, :]
        bx_h = bx[h * DIM_HEAD:(h + 1) * DIM_HEAD]
        wlg_blocks.append((Wslice @ Wx_h) / tau[h])
        blg_blocks.append((Wslice @ bx_h + bslice) / tau[h])
    wlgT = np.ascontiguousarray(np.concatenate(wlg_blocks, 0).T, f32)
    blg = np.concatenate(blg_blocks, 0).reshape(1, INNER).astype(f32)
    scale = DIM_HEAD ** -0.5
    return {
        "wfxT": np.ascontiguousarray(Wfx.T).astype(BF_NP),
        "wlgT": wlgT.astype(BF_NP),
        "blg": blg.astype(BF_NP),
        "onesb": np.ones((1, P), BF_NP),
        "bfxb": _bfx_pair(bfx),
        "wqT": np.ascontiguousarray((Wq * scale).T, f32),
        "wkT": np.ascontiguousarray(Wk.T, f32),
        "wvT": np.ascontiguousarray(Wv.T, f32),
        "woT": np.ascontiguousarray(
            Wout.T.reshape(HEADS, DIM_HEAD, DIM).transpose(1, 0, 2), f32),
        "bout8b": np.ascontiguousarray(
            np.tile(bout[None, :] / 8.0, (P, 1)), f32),
        "idf32": np.eye(P, dtype=np.float32),
    }


_PROG = {}


def _get_prog(nshard, dbg=False):
    if (nshard, dbg) not in _PROG:
        _PROG[(nshard, dbg)] = build_program(nshard, dbg)
    return _PROG[(nshard, dbg)]


def run(inputs, nshard=NSHARD, trace=False, trace_cores=None, dbg=False):
    x = np.asarray(inputs["x"], np.float32)
    b_, n_, d_ = x.shape
    assert d_ == DIM and n_ == 2 * nshard and b_ == B
    nc = _get_prog(nshard, dbg)
    common = prep_weights(inputs)
    in_maps = []
    for core in range(NCORES):
        bb, half = core // 2, core % 2
        xs = x[bb, half * nshard:(half + 1) * nshard, :]
        m = dict(common)
        m["xT"] = np.ascontiguousarray(xs.T).astype(BF_NP)
        in_maps.append(m)
    res = run_bass_kernel_spmd(nc, in_maps, list(range(NCORES)),
                               trace=trace, trace_cores=trace_cores)
    full = np.empty((B, n_, DIM), np.float32)
    for core in range(NCORES):
        bb, half = core // 2, core % 2
        full[bb, half * nshard:(half + 1) * nshard, :] = \
            res.results[core]["out"]
    return full, res


def kernel(**inputs):
    out, _ = run(inputs)
    return out


# revision 35
# speedup vs baseline: 1.1994x; 1.1147x over previous
import sys, os
for _p in ("/opt/trn_rl_repo",):
    if _p not in sys.path:
        sys.path.append(_p)

import numpy as np
import ml_dtypes
from contextlib import ExitStack

import concourse.bass as bass
import concourse.bacc as bacc
import concourse.tile as tile
from concourse import mybir
from concourse.bass_utils import run_bass_kernel_spmd

F32 = mybir.dt.float32
BF16 = mybir.dt.bfloat16
BF_NP = ml_dtypes.bfloat16

DIM = 256
HEADS = 8
DIM_HEAD = 64
SLICE_NUM = 64
INNER = HEADS * DIM_HEAD  # 512
B, N = 4, 32768
NCORES = 8
NSHARD = N // 2  # 16384 tokens per core
P = 128
EXPF = mybir.ActivationFunctionType.Exp


def build_program(nshard, dbg=False):
    NT = nshard // P
    assert NT % 2 == 0
    nc = bacc.Bacc("TRN2", target_bir_lowering=False, debug=False,
                   num_devices=NCORES)
    if dbg:
        dbg_pooled = nc.dram_tensor("dbg_pooled", [P, 4, 130], F32,
                                    kind="ExternalOutput").ap()
        dbg_m2 = nc.dram_tensor("dbg_m2", [P, 4, DIM], BF16,
                                kind="ExternalOutput").ap()
        dbg_wT = nc.dram_tensor("dbg_wT", [P, 4, nshard], BF16,
                                kind="ExternalOutput").ap()
    xT_h = nc.dram_tensor("xT", [DIM, nshard], BF16, kind="ExternalInput")
    wfxT = nc.dram_tensor("wfxT", [DIM, INNER], BF16, kind="ExternalInput").ap()
    wlgT = nc.dram_tensor("wlgT", [DIM, INNER], BF16, kind="ExternalInput").ap()
    blg = nc.dram_tensor("blg", [1, INNER], BF16, kind="ExternalInput").ap()
    onesb = nc.dram_tensor("onesb", [1, P], BF16, kind="ExternalInput").ap()
    bfxb = nc.dram_tensor("bfxb", [P, 4, 64], F32, kind="ExternalInput").ap()
    wqT = nc.dram_tensor("wqT", [64, 64], F32, kind="ExternalInput").ap()
    wkT = nc.dram_tensor("wkT", [64, 64], F32, kind="ExternalInput").ap()
    wvT = nc.dram_tensor("wvT", [64, 64], F32, kind="ExternalInput").ap()
    woT = nc.dram_tensor("woT", [64, HEADS, DIM], F32, kind="ExternalInput").ap()
    bout8b = nc.dram_tensor("bout8b", [P, DIM], F32, kind="ExternalInput").ap()
    idf32 = nc.dram_tensor("idf32", [P, P], F32, kind="ExternalInput").ap()
    out_h = nc.dram_tensor("out", [nshard, DIM], F32, kind="ExternalOutput")
    out_ap = out_h.ap()

    with tile.TileContext(nc) as tc, ExitStack() as ctx:
        cpool = ctx.enter_context(tc.tile_pool(name="consts", bufs=1))
        big = ctx.enter_context(tc.tile_pool(name="big", bufs=1))

        # big weights on the scalar queue so x tiles start on sync at once
        wfx_sb = cpool.tile([P, 2, INNER], BF16)
        wlg_sb = cpool.tile([P, 2, INNER], BF16)
        for c in range(2):
            nc.scalar.dma_start(wfx_sb[:, c, :], wfxT[c * P:(c + 1) * P, :])
            nc.scalar.dma_start(wlg_sb[:, c, :], wlgT[c * P:(c + 1) * P, :])
        blg_sb = cpool.tile([1, INNER], BF16)
        nc.scalar.dma_start(blg_sb[:], blg[:])
        ones1 = cpool.tile([1, P], BF16)
        nc.scalar.dma_start(ones1[:], onesb[:])
        bfx_sb = cpool.tile([P, 4, 64], F32)
        nc.scalar.dma_start(bfx_sb[:], bfxb[:])
        wq_sb = cpool.tile([64, 64], F32)
        wk_sb = cpool.tile([64, 64], F32)
        wv_sb = cpool.tile([64, 64], F32)
        nc.scalar.dma_start(wq_sb[:], wqT[:])
        nc.scalar.dma_start(wk_sb[:], wkT[:])
        nc.scalar.dma_start(wv_sb[:], wvT[:])
        wo_sb = cpool.tile([64, HEADS, DIM], F32)
        nc.scalar.dma_start(wo_sb[:], woT[:])
        bout8_sb = cpool.tile([P, DIM], F32)
        nc.scalar.dma_start(bout8_sb[:], bout8b[:])
        idf_sb = cpool.tile([P, P], F32)
        nc.scalar.dma_start(idf_sb[:], idf32[:])

        # persistent across phases
        # transposed slice weights, blocked: [g, t4, (t%4)*4+c, tok]
        wT_sb = big.tile([P, nshard // (4 * P), 16, P], BF16)
        pooled_sb = big.tile([P, 4, 130], F32)   # after allreduce
        m2_sb = big.tile([P, 4, DIM], BF16)      # out_slice @ WoutT per hg
        # manual 6-slot fx staging; ones cols preset once (norm columns)
        FXS = 6
        fx2_sb = big.tile([P, FXS, 4, 130], BF16)
        nc.vector.memset(fx2_sb[:, :, :, 128:130], 1.0)
        # 12-slot w staging ring, consumed by pools + 4-wide batched transpose
        WS = 12
        w8_sb = big.tile([P, WS, HEADS, SLICE_NUM], BF16)

        # ---------------- pass 1 ----------------
        # software-pipelined: pool matmuls + wT transpose for sub-tile t are
        # emitted DLY iterations late so the PE/sync queues never head-of-line
        # block on the exp->reduce->recip->mul chain.
        DLY = 4
        XB = 4  # sub-tiles per x DMA
        with tc.tile_pool(name="xp", bufs=3) as xpool, \
             tc.tile_pool(name="sp", bufs=8) as spool, \
             tc.tile_pool(name="fxps", bufs=3, space="PSUM") as fxps, \
             tc.tile_pool(name="lgps", bufs=3, space="PSUM") as lgps, \
             tc.tile_pool(name="poolps", bufs=1, space="PSUM") as poolps:
            # two accumulators per bank; start=True resets per-address, so
            # disjoint column ranges in one bank are safe
            pool_ps = [poolps.tile([P, 2, 130], F32, name=f"pool_ps{i}")
                       for i in range(2)]

            def emit_late(u):
                for q in range(4):
                    nc.tensor.matmul(pool_ps[q // 2][:, q % 2, :],
                                     w8_sb[:, u % WS, 2 * q:2 * q + 2, :],
                                     fx2_sb[:, u % FXS, q, :],
                                     start=(u == 0), stop=(u == NT - 1))
                if u % 4 == 3:
                    # one blocked DMA transpose for 4 sub-tiles:
                    # wT[g, (t',c), tok] = w[tok, (t',c)*128+g]
                    b = u // 4
                    s0 = (b % (WS // 4)) * 4
                    nc.sync.dma_start_transpose(
                        wT_sb[:, b, :, :], w8_sb[:, s0:s0 + 4, :, :])

            # warm the PE clock before the steady-state stream (scratch
            # writes into pool_ps[0]; the real accumulation's start=True at
            # t==0 overwrites them)
            for _ in range(15):
                nc.tensor.matmul(pool_ps[0][:], wfx_sb[:, 0, 0:128],
                                 wfx_sb[:, 0, 0:260], start=True, stop=True)

            for t in range(NT):
                if t % XB == 0:
                    xt = xpool.tile([P, 2, XB * P], BF16)
                    src = bass.AP(xT_h, t * P,
                                  [[nshard, P], [P * nshard, 2], [1, XB * P]])
                    nc.sync.dma_start(xt[:], src)
                s = t % XB
                xa = xt[:, 0, s * P:(s + 1) * P]
                xb = xt[:, 1, s * P:(s + 1) * P]
                fxp = fxps.tile([P, 4, P], F32)
                nc.tensor.matmul(fxp[:], xa, wfx_sb[:, 0, :],
                                 start=True, stop=False)
                nc.tensor.matmul(fxp[:], xb, wfx_sb[:, 1, :],
                                 start=False, stop=True)
                lgp = lgps.tile([P, HEADS, SLICE_NUM], F32)
                nc.tensor.matmul(lgp[:], ones1[:], blg_sb[:],
                                 start=True, stop=False)
                nc.tensor.matmul(lgp[:], xa, wlg_sb[:, 0, :],
                                 start=False, stop=False)
                nc.tensor.matmul(lgp[:], xb, wlg_sb[:, 1, :],
                                 start=False, stop=True)
                # softmax over slices (bounded logits: skip max-sub)
                nc.scalar.copy(fx2_sb[:, t % FXS, 0:2, 0:128], fxp[:, 0:2, :])
                e_t = spool.tile([P, HEADS, SLICE_NUM], BF16)
                nc.scalar.activation(e_t[:], lgp[:], EXPF)
                s_t = spool.tile([P, HEADS], F32)
                nc.vector.tensor_copy(fx2_sb[:, t % FXS, 2:4, 0:128],
                                      fxp[:, 2:4, :])
                nc.vector.tensor_reduce(s_t[:], e_t[:],
                                        axis=mybir.AxisListType.X,
                                        op=mybir.AluOpType.add)
                r_t = spool.tile([P, HEADS], F32)
                nc.vector.reciprocal(r_t[:], s_t[:])
                nc.gpsimd.tensor_mul(
                    w8_sb[:, t % WS, :, :], e_t[:],
                    r_t[:, :, None].to_broadcast([P, HEADS, SLICE_NUM]))
                if t >= DLY:
                    emit_late(t - DLY)
            for u in range(NT - DLY, NT):
                emit_late(u)

            # -------- allreduce pooled sums over the token-half pair --------
            with tc.tile_pool(name="ccdram", bufs=1, space="DRAM") as dpool:
                b_in = dpool.tile([P, 4, 130], BF16)
                b_out = dpool.tile([P, 4, 130], BF16)
                pre_sb = big.tile([P, 4, 130], BF16)
                nc.scalar.copy(pre_sb[:, 0:2, :], pool_ps[0][:])
                nc.vector.tensor_copy(pre_sb[:, 2:4, :], pool_ps[1][:])
                nc.sync.dma_start(b_in[:], pre_sb[:])
                nc.gpsimd.collective_compute(
                    "AllReduce", mybir.AluOpType.add,
                    replica_groups=[[0, 1], [2, 3], [4, 5], [6, 7]],
                    ins=[b_in.opt()], outs=[b_out.opt()])
                pooled_bf = big.tile([P, 4, 130], BF16)
                nc.sync.dma_start(pooled_bf[:], b_out[:])
                nc.vector.tensor_copy(pooled_sb[:], pooled_bf[:])

        # ---------------- tiny slice attention (head pairs, 128-wide) ----
        with tc.tile_pool(name="mps", bufs=1, space="PSUM") as mps, \
             tc.tile_pool(name="msb", bufs=2) as msb:
            for q4 in range(4):
                # gather diagonal S blocks: row j*64+g <- own head's channels
                gath = msb.tile([P, 64], F32)
                for j in range(2):
                    nc.sync.dma_start(
                        gath[j * 64:(j + 1) * 64, :],
                        pooled_sb[j * 64:(j + 1) * 64, q4, j * 64:j * 64 + 64])
                norm = pooled_sb[:, q4, 128:129]
                nrm = msb.tile([P, 1], F32)
                nc.vector.tensor_scalar_add(nrm[:], norm, 1e-5)
                rho = msb.tile([P, 1], F32)
                nc.vector.reciprocal(rho[:], nrm[:])
                tmp = msb.tile([P, 64], F32)
                nc.vector.tensor_scalar_mul(tmp[:], bfx_sb[:, q4, :], norm)
                stp = msb.tile([P, 64], F32)
                nc.vector.tensor_add(stp[:], gath[:], tmp[:])
                st = msb.tile([P, 64], F32)
                nc.vector.tensor_scalar_mul(st[:], stp[:], rho[:])
                # stT [c, j*64+g] = slice_token[head j, g, c]
                stT_p = mps.tile([64, P], F32)
                nc.tensor.transpose(stT_p[:], st[:], idf_sb[:])
                stT = msb.tile([64, P], F32)
                nc.scalar.copy(stT[:], stT_p[:])
                # q^T, k^T for both heads at once: [o, j*64+g]
                qk_p = mps.tile([64, 2, P], F32)
                nc.tensor.matmul(qk_p[:, 0, :], wq_sb[:], stT[:],
                                 start=True, stop=True)
                nc.tensor.matmul(qk_p[:, 1, :], wk_sb[:], stT[:],
                                 start=True, stop=True)
                qk = msb.tile([64, 2, P], F32)
                nc.scalar.copy(qk[:], qk_p[:])
                # logits per head -> stacked [j*64+g, g']
                L_p = mps.tile([P, 64], F32)
                for j in range(2):
                    nc.tensor.matmul(L_p[j * 64:(j + 1) * 64, :],
                                     qk[:, 0, j * 64:(j + 1) * 64],
                                     qk[:, 1, j * 64:(j + 1) * 64],
                                     start=True, stop=True)
                ea = msb.tile([P, 64], F32)
                srow = msb.tile([P, 1], F32)
                nc.scalar.activation(ea[:], L_p[:], EXPF, accum_out=srow[:])
                rha = msb.tile([P, 1], F32)
                nc.vector.reciprocal(rha[:], srow[:])
                attn = msb.tile([P, 64], F32)
                nc.vector.tensor_scalar_mul(attn[:], ea[:], rha[:])
                # aT [g, j*64+g'] = attn[head j, g', g]
                aT_p = mps.tile([64, P], F32)
                nc.tensor.transpose(aT_p[:], attn[:], idf_sb[:])
                aT = msb.tile([64, P], F32)
                nc.scalar.copy(aT[:], aT_p[:])
                # v per head [g, o] (base partition 0), then os = attn @ v
                os_p = mps.tile([P, 64], F32)
                for j in range(2):
                    v_p = mps.tile([64, 64], F32)
                    nc.tensor.matmul(v_p[:], stT[:, j * 64:(j + 1) * 64],
                                     wv_sb[:], start=True, stop=True)
                    v_sb = msb.tile([64, 64], F32)
                    nc.scalar.copy(v_sb[:], v_p[:])
                    nc.tensor.matmul(os_p[j * 64:(j + 1) * 64, :],
                                     aT[:, j * 64:(j + 1) * 64], v_sb[:],
                                     start=True, stop=True)
                os_sb = msb.tile([P, 64], F32)
                nc.scalar.copy(os_sb[:], os_p[:])
                osT_p = mps.tile([64, P], F32)
                nc.tensor.transpose(osT_p[:], os_sb[:], idf_sb[:])
                osT = msb.tile([64, P], F32)
                nc.scalar.copy(osT[:], osT_p[:])
                m2_p = mps.tile([P, DIM], F32)
                for j in range(2):
                    nc.tensor.matmul(m2_p[j * 64:(j + 1) * 64, :],
                                     osT[:, j * 64:(j + 1) * 64],
                                     wo_sb[:, 2 * q4 + j, :],
                                     start=True, stop=True)
                # fold bout/8 into m2 (softmax weights sum to 8 over 512 g)
                nc.vector.tensor_add(m2_sb[:, q4, :], m2_p[:], bout8_sb[:])

        # ---------------- pass 2: unpool + output proj ----------------
        with tc.tile_pool(name="p2ps", bufs=8, space="PSUM") as p2ps, \
             tc.tile_pool(name="p2sb", bufs=5) as p2sb:
            for t4 in range(NT // 4):
                ob4 = p2sb.tile([P, 4, DIM], F32)
                for k in range(4):
                    t = 4 * t4 + k
                    op = p2ps.tile([P, DIM], F32)
                    for c in range(4):
                        nc.tensor.matmul(
                            op[:], wT_sb[:, t4, k * 4 + c, :],
                            m2_sb[:, c, :],
                            start=(c == 0), stop=(c == 3))
                    if k % 2 == 0:
                        nc.vector.tensor_copy(ob4[:, k, :], op[:])
                    else:
                        nc.scalar.copy(ob4[:, k, :], op[:])
                # one quad DMA: dst rows t4*512 + k*128 + p
                dst = bass.AP(out_h, t4 * 4 * P * DIM,
                              [[DIM, P], [P * DIM, 4], [1, DIM]])
                nc.sync.dma_start(dst, ob4[:])
        if dbg:
            nc.sync.dma_start(dbg_pooled[:], pooled_sb[:])
            nc.sync.dma_start(dbg_m2[:], m2_sb[:])
            nc.sync.dma_start(dbg_wT[:], wT_sb[:])
    nc.compile()
    return nc


def _bfx_pair(bfx):
    bfx2 = bfx.reshape(HEADS, DIM_HEAD)
    out = np.empty((P, 4, 64), np.float32)
    for q4 in range(4):
        for j in range(2):
            out[j * 64:(j + 1) * 64, q4, :] = bfx2[2 * q4 + j]
    return out


def prep_weights(inputs):
    f32 = np.float32
    Wfx = np.asarray(inputs["Wfx"], f32)
    bfx = np.asarray(inputs["bfx"], f32)
    Wx = np.asarray(inputs["Wx"], f32)
    bx = np.asarray(inputs["bx"], f32)
    Wslice = np.asarray(inputs["Wslice"], f32)
    bslice = np.asarray(inputs["bslice"], f32)
    tau = np.asarray(inputs["temperature"], f32).reshape(HEADS)
    Wq = np.asarray(inputs["Wq"], f32)
    Wk = np.asarray(inputs["Wk"], f32)
    Wv = np.asarray(inputs["Wv"], f32)
    Wout = np.asarray(inputs["Wout"], f32)
    bout = np.asarray(inputs["bout"], f32)

    wlg_blocks = []
    blg_blocks = []
    for h in range(HEADS):
        Wx_h = Wx[h * DIM_HEAD:(h + 1) * DIM_HEAD, :]
        bx_h = bx[h * DIM_HEAD:(h + 1) * DIM_HEAD]
        wlg_blocks.append((Wslice @ Wx_h) / tau[h])
        blg_blocks.append((Wslice @ bx_h + bslice) / tau[h])
    wlgT = np.ascontiguousarray(np.concatenate(wlg_blocks, 0).T, f32)
    blg = np.concatenate(blg_blocks, 0).reshape(1, INNER).astype(f32)
    scale = DIM_HEAD ** -0.5
    return {
        "wfxT": np.ascontiguousarray(Wfx.T).astype(BF_NP),
        "wlgT": wlgT.astype(BF_NP),
        "blg": blg.astype(BF_NP),
        "onesb": np.ones((1, P), BF_NP),
        "bfxb": _bfx_pair(bfx),
        "wqT": np.ascontiguousarray((Wq * scale).T, f32),
        "wkT": np.ascontiguousarray(Wk.T, f32),
        "wvT": np.ascontiguousarray(Wv.T, f32),
        "woT": np.ascontiguousarray(
            Wout.T.reshape(HEADS, DIM_HEAD, DIM).transpose(1, 0, 2), f32),
        "bout8b": np.ascontiguousarray(
            np.tile(bout[None, :] / 8.0, (P, 1)), f32),
        "idf32": np.eye(P, dtype=np.float32),
    }


_PROG = {}


def _get_prog(nshard, dbg=False):
    if (nshard, dbg) not in _PROG:
        _PROG[(nshard, dbg)] = build_program(nshard, dbg)
    return _PROG[(nshard, dbg)]


def run(inputs, nshard=NSHARD, trace=False, trace_cores=None, dbg=False):
    x = np.asarray(inputs["x"], np.float32)
    b_, n_, d_ = x.shape
    assert d_ == DIM and n_ == 2 * nshard and b_ == B
    nc = _get_prog(nshard, dbg)
    common = prep_weights(inputs)
    in_maps = []
    for core in range(NCORES):
        bb, half = core // 2, core % 2
        xs = x[bb, half * nshard:(half + 1) * nshard, :]
        m = dict(common)
        m["xT"] = np.ascontiguousarray(xs.T).astype(BF_NP)
        in_maps.append(m)
    res = run_bass_kernel_spmd(nc, in_maps, list(range(NCORES)),
                               trace=trace, trace_cores=trace_cores)
    full = np.empty((B, n_, DIM), np.float32)
    for core in range(NCORES):
        bb, half = core // 2, core % 2
        full[bb, half * nshard:(half + 1) * nshard, :] = \
            res.results[core]["out"]
    return full, res


def kernel(**inputs):
    out, _ = run(inputs)
    return out
